# revision 11
# baseline (speedup 1.0000x reference)
"""STFT (n_fft=4096, hop=1024, centered reflect-pad, Hann) on 8 TRN2 cores.

Algorithm: 2-stage Cooley-Tukey, n = 128*n1 + n2 (n1 in [0,32), n2 in [0,128)),
k = k1 + 32*k2 (k1 in [0,32), k2 in [0,64] for the 2049 kept bins).

  X[k1+32k2, b] = sum_n2 G[n2,k] * sum_n1 e^{-2pi i n1 k1/32} * xw[b, 128n1+n2]

Stage 1 runs frames-as-weights so its output lands transposed (n2 on
partitions): per 4-frame subgroup one [128,128] fp16 lhsT (4 frames
interleaved across partitions) against a constant one-hot-structured rhs
[128,256]. Stage 2 contracts n2 (K=128) with per-k1 twiddle matrices in fp16
over B=256-frame groups (N=256 matmuls).

DMA layout: the host pre-windows + pre-gathers the stage-1 lhsT content into
xfr[2, 128, 129*128] fp16 (partition p = 32j+8r+i, cols = 128*subgroup + m),
so every input DMA is a plain 2D tile load with multi-KB contiguous
per-partition runs (the fp32 4-copy scheme moved everything in 512B packets).
Output is written in SBUF order to o[128, 16512] fp16 (partition = 64c+p,
cols = 32*gb0 + q*B + b, bin = 32p+q) and unscrambled on the host.

Sharding: frame-parallel. Core i computes 516 frames starting at frame 512*i
(SPMD, same NEFF); host trims/concatenates to the 4097 global frames.
"""

import numpy as np

import concourse.bacc as bacc
import concourse.tile as tile
import concourse.mybir as mybir
from concourse import bass_utils

N_FFT = 4096
HOP = 1024
T = 4194304
NBINS = N_FFT // 2 + 1          # 2049
F_TOTAL = T // HOP + 1          # 4097
NCORES = 8

NF = 516                        # frames computed per core (129 subgroups of 4)
NSG = NF // 4                   # 129 subgroups
GROUPS = [260, 256]
L = (NF - 1) * HOP + N_FFT      # per-core input samples per plane = 531456

F32 = mybir.dt.float32
F16 = mybir.dt.float16

_cache = {}
LAST_EXEC_NS = None
LAST_RES = None


def _host_constants():
    n1 = np.arange(32)
    k1 = np.arange(32)
    C = np.cos(2 * np.pi * np.outer(n1, k1) / 32)
    S = np.sin(2 * np.pi * np.outer(n1, k1) / 32)
    R1 = np.concatenate([C, -S], axis=1)      # [n1, 64]
    R2 = np.concatenate([S, C], axis=1)
    # lhsT partition p = 32j + 8r + i  <->  (n1 = 8j+i, frame r)
    R1D = np.zeros((128, 256), np.float16)
    R2D = np.zeros((128, 256), np.float16)
    for j in range(4):
        for i in range(8):
            for r in range(4):
                p = 32 * j + 8 * r + i
                R1D[p, 64 * r:64 * r + 64] = R1[8 * j + i]
                R2D[p, 64 * r:64 * r + 64] = R2[8 * j + i]

    n2 = np.arange(128)
    k2 = np.arange(64)
    Gp = np.zeros((128, 32 * 128), np.float16)
    Gq = np.zeros((128, 32 * 128), np.float16)
    for q in range(32):
        kk = q + 32 * k2
        ang = 2 * np.pi * np.outer(n2, kk) / N_FFT
        gr = np.cos(ang)
        gi = -np.sin(ang)
        Gp[:, 128 * q:128 * q + 64] = gr.astype(np.float16)
        Gp[:, 128 * q + 64:128 * q + 128] = gi.astype(np.float16)
        Gq[:, 128 * q:128 * q + 64] = (-gi).astype(np.float16)
        Gq[:, 128 * q + 64:128 * q + 128] = gr.astype(np.float16)

    alt = ((-1.0) ** n2).astype(np.float16)
    E1 = np.zeros((128, 2), np.float16)
    E2 = np.zeros((128, 2), np.float16)
    E1[:, 0] = alt
    E2[:, 1] = alt
    return (R1D, R2D, Gp, Gq, E1, E2)


def _build(stages=("dma", "s1", "s2", "out")):
    stages = set(stages)
    nc = bacc.Bacc("TRN2", target_bir_lowering=False, debug=False,
                   enable_asserts=False, num_devices=NCORES)
    xfr = nc.dram_tensor("xfr", [2, 128, NSG * 128], F16, kind="ExternalInput")
    r1d = nc.dram_tensor("r1d", [128, 256], F16, kind="ExternalInput")
    r2d = nc.dram_tensor("r2d", [128, 256], F16, kind="ExternalInput")
    gp = nc.dram_tensor("gp", [128, 32 * 128], F16, kind="ExternalInput")
    gq = nc.dram_tensor("gq", [128, 32 * 128], F16, kind="ExternalInput")
    e1 = nc.dram_tensor("e1", [128, 2], F16, kind="ExternalInput")
    e2 = nc.dram_tensor("e2", [128, 2], F16, kind="ExternalInput")
    out = nc.dram_tensor("o", [128, 32 * NF], F16, kind="ExternalOutput")
    oute = nc.dram_tensor("oe", [2, NF], F16, kind="ExternalOutput")

    with tile.TileContext(nc) as tc:
        with (
            tc.tile_pool(name="const", bufs=1) as cpool,
            tc.tile_pool(name="fr", bufs=2) as frpool,
            tc.tile_pool(name="ys", bufs=2) as yspool,
            tc.tile_pool(name="ost", bufs=2) as ostpool,
            tc.tile_pool(name="ps1", bufs=3, space="PSUM") as ps1pool,
            tc.tile_pool(name="ps2", bufs=3, space="PSUM") as ps2pool,
            tc.tile_pool(name="pse", bufs=2, space="PSUM") as psepool,
        ):
            t_r1 = cpool.tile([128, 256], F16, tag="r1")
            t_r2 = cpool.tile([128, 256], F16, tag="r2")
            t_gp = cpool.tile([128, 32 * 128], F16, tag="gp")
            t_gq = cpool.tile([128, 32 * 128], F16, tag="gq")
            t_e1 = cpool.tile([128, 2], F16, tag="e1")
            t_e2 = cpool.tile([128, 2], F16, tag="e2")
            # r1/r2 (needed by stage 1) go first on the sync queue; the big
            # stage-2 constants ride the vector queue so the group-0 input
            # load isn't stuck behind them.
            nc.sync.dma_start(t_r1[:], r1d.ap()[:, :])
            nc.sync.dma_start(t_r2[:], r2d.ap()[:, :])
            nc.scalar.dma_start(t_gp[:], gp.ap()[:, :])
            nc.scalar.dma_start(t_gq[:], gq.ap()[:, :])
            nc.scalar.dma_start(t_e1[:], e1.ap()[:, :])
            nc.scalar.dma_start(t_e2[:], e2.ap()[:, :])

            starts = []
            gb0 = 0
            for B in GROUPS:
                starts.append(gb0)
                gb0 += B

            def emit_load_s1(gb0, B):
                nsub = B // 4
                sg0 = gb0 // 4
                ncols = 128 * nsub
                fr_r = frpool.tile([128, 128 * 65], F16, tag="fr_r")
                fr_i = frpool.tile([128, 128 * 65], F16, tag="fr_i")
                # plain 2D tile loads; chunked so stage-1 can start early
                nchunk = 4 if nsub >= 4 else 1
                step = nsub // nchunk
                for c, fr_t in ((0, fr_r), (1, fr_i)):
                    for ch in range(nchunk):
                        a = 128 * step * ch
                        b = 128 * step * (ch + 1) if ch < nchunk - 1 else ncols
                        nc.sync.dma_start(
                            fr_t[:, a:b],
                            xfr.ap()[c, :, 128 * sg0 + a:128 * sg0 + b])

                # ys layout: [p, 64*c + ... ] transposed — col = c*B + b so
                # stage-2 rhs slices are contiguous in b.
                ys = yspool.tile([128, 64 * 260], F16, tag="ys")
                ysT = ys[:, 0:64 * B].rearrange("p (c b) -> p b c", b=B)
                if "s1" not in stages:
                    return ys
                npair = (nsub + 1) // 2
                for sp in range(npair):
                    s0 = 2 * sp
                    nsg = min(2, nsub - s0)
                    w = 256 * nsg
                    ps1 = ps1pool.tile([128, 512], F32, tag="ps1")
                    for t in range(nsg):
                        s = s0 + t
                        cs = 256 * t
                        nc.tensor.matmul(ps1[:, cs:cs + 256],
                                         fr_r[:, 128 * s:128 * s + 128],
                                         t_r1[:], start=(t == 0), stop=False)
                        nc.tensor.matmul(ps1[:, cs:cs + 256],
                                         fr_i[:, 128 * s:128 * s + 128],
                                         t_r2[:], start=False,
                                         stop=(t == nsg - 1))
                    # ps1 col = 64*l + c (l = frame-in-pair), scatter to
                    # ys col = c*B + (8*sp + l)
                    src = ps1[:, 0:w].rearrange("p (l c) -> p l c", c=64)
                    dstc = ysT[:, 8 * sp:8 * sp + 4 * nsg, :]
                    if sp % 2 == 0:
                        nc.vector.tensor_copy(dstc, src)
                    else:
                        nc.scalar.copy(dstc, src)
                return ys

            def emit_s2_out(gb0, B, ys):
                if "s2" not in stages:
                    return
                ost = ostpool.tile([128, 32 * 260], F16, tag="ost")
                oc0 = 32 * gb0          # output col base for this group
                for q in range(32):
                    rhs_r = ys[:, B * q:B * q + B]
                    rhs_i = ys[:, B * (32 + q):B * (32 + q) + B]
                    ps2 = ps2pool.tile([128, 260], F32, tag="ps2")
                    nc.tensor.matmul(ps2[:, 0:B],
                                     t_gp[:, 128 * q:128 * q + 128],
                                     rhs_r, start=True, stop=False)
                    nc.tensor.matmul(ps2[:, 0:B],
                                     t_gq[:, 128 * q:128 * q + 128],
                                     rhs_i, start=False, stop=True)
                    d0 = ost[:, B * q:B * q + B]
                    if q % 2 == 0:
                        nc.vector.tensor_copy(d0, ps2[:, 0:B])
                    else:
                        nc.scalar.copy(d0, ps2[:, 0:B])
                    if "out" in stages and q == 15:
                        nc.scalar.dma_start(
                            out.ap()[:, oc0:oc0 + 16 * B],
                            ost[:, 0:16 * B])

                # bin 2048 (k1=0, k2=64)
                pse = psepool.tile([2, 260], F32, tag="pse")
                nc.tensor.matmul(pse[:, 0:B], t_e1[:], ys[:, 0:B],
                                 start=True, stop=False)
                nc.tensor.matmul(pse[:, 0:B], t_e2[:], ys[:, 32 * B:33 * B],
                                 start=False, stop=True)
                oste = ostpool.tile([2, 260], F16, tag="oste")
                nc.vector.tensor_copy(oste[:, 0:B], pse[:, 0:B])

                if "out" in stages:
                    nc.scalar.dma_start(
                        out.ap()[:, oc0 + 16 * B:oc0 + 32 * B],
                        ost[:, 16 * B:32 * B])
                    nc.scalar.dma_start(oute.ap()[:, gb0:gb0 + B],
                                        oste[:, 0:B])

            pending = None
            for gi, B in enumerate(GROUPS):
                ys = emit_load_s1(starts[gi], B)
                if pending is not None:
                    emit_s2_out(*pending)
                pending = (starts[gi], B, ys)
            emit_s2_out(*pending)

    nc.compile()
    return nc


def _prep_inputs(x, window):
    """Per-core stage-1 lhsT tensors: xfr[2, 128, 129*128] fp16 with
    partition p = 32j+8r+i holding frame-quarter j of frame 4*sg+r,
    cols = 128*sg + m, value = xp[1024*(b+j)+128i+m] * w[1024j+128i+m]."""
    pad = N_FFT // 2
    xp = np.pad(np.asarray(x), ((0, 0), (pad, pad)), mode="reflect")
    total = xp.shape[1]
    need = (NCORES - 1) * 512 * HOP + L
    xp_ext = np.zeros((2, max(total, need)), np.float32)
    xp_ext[:, :total] = xp
    w = np.asarray(window, np.float32)

    xfrs = []
    for i in range(NCORES):
        s0 = i * 512 * HOP
        seg = xp_ext[:, s0:s0 + L]
        xfr = np.empty((2, 128, NSG * 128), np.float16)
        for c in range(2):
            for j in range(4):
                Q = seg[c, 1024 * j:1024 * j + 1024 * NF].reshape(NF, 1024)
                Q = Q * w[1024 * j:1024 * (j + 1)][None, :]
                # [f, 1024] -> [sg, r, i, m] -> [r, i, sg, m]
                Q = Q.reshape(NSG, 4, 8, 128).transpose(1, 2, 0, 3)
                xfr[c, 32 * j:32 * j + 32] = \
                    Q.reshape(32, NSG * 128).astype(np.float16)
        xfrs.append(xfr)
    return xfrs


def kernel(x, window):
    import time
    t0 = time.time()
    x = np.asarray(x, np.float32)
    window = np.asarray(window, np.float32)
    if "nc" not in _cache:
        _cache["nc"] = _build()
    nc = _cache["nc"]
    print(f"[kernel] build done {time.time()-t0:.2f}s", flush=True)

    xfrs = _prep_inputs(x, window)
    R1D, R2D, Gp, Gq, E1, E2 = _host_constants()

    in_maps = []
    for i in range(NCORES):
        in_maps.append({"xfr": xfrs[i], "r1d": R1D, "r2d": R2D,
                        "gp": Gp, "gq": Gq, "e1": E1, "e2": E2})

    print(f"[kernel] inputs prepped {time.time()-t0:.2f}s", flush=True)
    res = bass_utils.run_bass_kernel_spmd(nc, in_maps,
                                          core_ids=list(range(NCORES)))
    print(f"[kernel] spmd done {time.time()-t0:.2f}s", flush=True)
    global LAST_EXEC_NS, LAST_RES
    LAST_RES = res
    if res.exec_time_ns is not None:
        LAST_EXEC_NS = res.exec_time_ns
        print(f"[kernel] exec_time_ns={res.exec_time_ns}", flush=True)
        if res.instructions_and_trace is not None:
            print(f"[kernel] trace={res.instructions_and_trace[1]}",
                  flush=True)

    out = np.zeros((2, NBINS, F_TOTAL), np.float32)
    for i in range(NCORES):
        o = res.results[i]["o"]            # [128, 32*NF] fp16
        oe = res.results[i]["oe"]          # [2, NF] fp16
        f0 = 512 * i
        nf = 513 if i == NCORES - 1 else 512
        full = np.empty((2, 2048, NF), np.float32)
        gb0 = 0
        for B in GROUPS:
            seg = o[:, 32 * gb0:32 * gb0 + 32 * B].astype(np.float32)
            # [128, 32*B] -> [c, p, q, b] -> [c, 32p+q, b]
            full[:, :, gb0:gb0 + B] = \
                seg.reshape(2, 64, 32, B).reshape(2, 2048, B)
            gb0 += B
        out[:, :2048, f0:f0 + nf] = full[:, :, :nf]
        out[:, 2048, f0:f0 + nf] = oe[:, :nf].astype(np.float32)
    return out


# revision 19
# speedup vs baseline: 1.6081x; 1.6081x over previous
"""STFT (n_fft=4096, hop=1024, centered reflect-pad, Hann) on 8 TRN2 cores.

Algorithm: 2-stage Cooley-Tukey, n = 128*n1 + n2 (n1 in [0,32), n2 in [0,128)),
k = k1 + 32*k2 (k1 in [0,32), k2 in [0,64] for the 2049 kept bins).

  X[k1+32k2, b] = sum_n2 G[n2,k] * sum_n1 e^{-2pi i n1 k1/32} * xw[b, 128n1+n2]

Stage 1 runs frames-as-weights so its output lands transposed (n2 on
partitions): per 4-frame subgroup one [128,128] fp16 lhsT (4 frames
interleaved across partitions) against a constant one-hot-structured rhs
[128,256]. Stage 2 contracts n2 (K=128) with per-k1 twiddle matrices in fp16
over B=256-frame groups (N=256 matmuls).

DMA layout: the host pre-windows + pre-gathers the stage-1 lhsT content into
xfr[2, 128, 129*128] fp16 (partition p = 32j+8r+i, cols = 128*subgroup + m),
so every input DMA is a plain 2D tile load with multi-KB contiguous
per-partition runs (the fp32 4-copy scheme moved everything in 512B packets).
Output is written in SBUF order to o[128, 16512] fp16 (partition = 64c+p,
cols = 32*gb0 + q*B + b, bin = 32p+q) and unscrambled on the host.

Sharding: frame-parallel. Core i computes 516 frames starting at frame 512*i
(SPMD, same NEFF); host trims/concatenates to the 4097 global frames.
"""

import numpy as np

import concourse.bacc as bacc
import concourse.tile as tile
import concourse.mybir as mybir
from concourse import bass_utils

N_FFT = 4096
HOP = 1024
T = 4194304
NBINS = N_FFT // 2 + 1          # 2049
F_TOTAL = T // HOP + 1          # 4097
NCORES = 8

NF = 516                        # frames computed per core (129 subgroups of 4)
NSG = NF // 4                   # 129 subgroups
GROUPS = [260, 256]
L = (NF - 1) * HOP + N_FFT      # per-core input samples per plane = 531456

F32 = mybir.dt.float32
F16 = mybir.dt.float16

_cache = {}
LAST_EXEC_NS = None
LAST_RES = None


def _host_constants():
    n1 = np.arange(32)
    k1 = np.arange(32)
    C = np.cos(2 * np.pi * np.outer(n1, k1) / 32)
    S = np.sin(2 * np.pi * np.outer(n1, k1) / 32)
    R1 = np.concatenate([C, -S], axis=1)      # [n1, 64]
    R2 = np.concatenate([S, C], axis=1)
    # lhsT partition p = 32j + 8r + i  <->  (n1 = 8j+i, frame r)
    # column order (c, r): col = 4*c + r, so stage-1 PSUM comes out
    # slot-major and the PSUM->SBUF copy writes contiguous frame runs.
    R1D = np.zeros((128, 256), np.float16)
    R2D = np.zeros((128, 256), np.float16)
    for j in range(4):
        for i in range(8):
            for r in range(4):
                p = 32 * j + 8 * r + i
                R1D[p, r::4] = R1[8 * j + i]
                R2D[p, r::4] = R2[8 * j + i]

    n2 = np.arange(128)
    k2 = np.arange(64)
    Gp = np.zeros((128, 32 * 128), np.float16)
    Gq = np.zeros((128, 32 * 128), np.float16)
    for q in range(32):
        kk = q + 32 * k2
        ang = 2 * np.pi * np.outer(n2, kk) / N_FFT
        gr = np.cos(ang)
        gi = -np.sin(ang)
        Gp[:, 128 * q:128 * q + 64] = gr.astype(np.float16)
        Gp[:, 128 * q + 64:128 * q + 128] = gi.astype(np.float16)
        Gq[:, 128 * q:128 * q + 64] = (-gi).astype(np.float16)
        Gq[:, 128 * q + 64:128 * q + 128] = gr.astype(np.float16)

    alt = ((-1.0) ** n2).astype(np.float16)
    E1 = np.zeros((128, 2), np.float16)
    E2 = np.zeros((128, 2), np.float16)
    E1[:, 0] = alt
    E2[:, 1] = alt
    return (R1D, R2D, Gp, Gq, E1, E2)


def _build(stages=("dma", "s1", "s2", "out")):
    stages = set(stages)
    nc = bacc.Bacc("TRN2", target_bir_lowering=False, debug=False,
                   enable_asserts=False, num_devices=NCORES)
    xfr = nc.dram_tensor("xfr", [2, 128, NSG * 128], F16, kind="ExternalInput")
    r1d = nc.dram_tensor("r1d", [128, 256], F16, kind="ExternalInput")
    r2d = nc.dram_tensor("r2d", [128, 256], F16, kind="ExternalInput")
    gp = nc.dram_tensor("gp", [128, 32 * 128], F16, kind="ExternalInput")
    gq = nc.dram_tensor("gq", [128, 32 * 128], F16, kind="ExternalInput")
    e1 = nc.dram_tensor("e1", [128, 2], F16, kind="ExternalInput")
    e2 = nc.dram_tensor("e2", [128, 2], F16, kind="ExternalInput")
    out = nc.dram_tensor("o", [128, 32 * NF], F16, kind="ExternalOutput")
    oute = nc.dram_tensor("oe", [2, NF], F16, kind="ExternalOutput")

    with tile.TileContext(nc) as tc:
        with (
            tc.tile_pool(name="const", bufs=1) as cpool,
            tc.tile_pool(name="fr", bufs=2) as frpool,
            tc.tile_pool(name="ys", bufs=2) as yspool,
            tc.tile_pool(name="ost", bufs=2) as ostpool,
            tc.tile_pool(name="ps1", bufs=2, space="PSUM") as ps1pool,
            tc.tile_pool(name="ps2", bufs=3, space="PSUM") as ps2pool,
            tc.tile_pool(name="pse", bufs=1, space="PSUM") as psepool,
        ):
            t_r1 = cpool.tile([128, 256], F16, tag="r1")
            t_r2 = cpool.tile([128, 256], F16, tag="r2")
            t_gp = cpool.tile([128, 32 * 128], F16, tag="gp")
            t_gq = cpool.tile([128, 32 * 128], F16, tag="gq")
            t_e1 = cpool.tile([128, 2], F16, tag="e1")
            t_e2 = cpool.tile([128, 2], F16, tag="e2")
            # r1/r2 (needed by stage 1) go first on the sync queue; the big
            # stage-2 constants ride the vector queue so the group-0 input
            # load isn't stuck behind them.
            nc.sync.dma_start(t_r1[:], r1d.ap()[:, :])
            nc.sync.dma_start(t_r2[:], r2d.ap()[:, :])
            nc.scalar.dma_start(t_gp[:], gp.ap()[:, :])
            nc.scalar.dma_start(t_gq[:], gq.ap()[:, :])
            nc.scalar.dma_start(t_e1[:], e1.ap()[:, :])
            nc.scalar.dma_start(t_e2[:], e2.ap()[:, :])

            starts = []
            gb0 = 0
            for B in GROUPS:
                starts.append(gb0)
                gb0 += B

            def emit_load_s1(gb0, B):
                nsub = B // 4
                sg0 = gb0 // 4
                ncols = 128 * nsub
                fr_r = frpool.tile([128, 128 * 65], F16, tag="fr_r")
                fr_i = frpool.tile([128, 128 * 65], F16, tag="fr_i")
                # plain 2D tile loads; chunked + plane-interleaved so
                # stage-1 can start after the first r/i chunk pair.
                nchunk = 8 if nsub >= 8 else 1
                step = nsub // nchunk
                for ch in range(nchunk):
                    a = 128 * step * ch
                    b = 128 * step * (ch + 1) if ch < nchunk - 1 else ncols
                    for c, fr_t in ((0, fr_r), (1, fr_i)):
                        nc.sync.dma_start(
                            fr_t[:, a:b],
                            xfr.ap()[c, :, 128 * sg0 + a:128 * sg0 + b])

                # ys layout: col = c*B + b (slot-major) so stage-2 rhs
                # slices are contiguous in b.
                ys = yspool.tile([128, 64 * 260], F16, tag="ys")
                ys_v = ys[:, 0:64 * B].rearrange("p (c b) -> p c b", b=B)
                if "s1" not in stages:
                    return ys
                nblk = (nsub + 3) // 4
                for blk in range(nblk):
                    s0 = 4 * blk
                    ns = min(4, nsub - s0)
                    ps1 = ps1pool.tile([128, 1024], F32, tag="ps1")
                    for t in range(ns):
                        s = s0 + t
                        cs = 256 * t
                        nc.tensor.matmul(ps1[:, cs:cs + 256],
                                         fr_r[:, 128 * s:128 * s + 128],
                                         t_r1[:], start=True, stop=False)
                        nc.tensor.matmul(ps1[:, cs:cs + 256],
                                         fr_i[:, 128 * s:128 * s + 128],
                                         t_r2[:], start=False, stop=True)
                    # ps1 col = 256*s_local + 4*c + r; ys col = c*B + b,
                    # b = 16*blk + 4*s_local + r: 32B-contiguous dst runs
                    src = ps1[:, 0:256 * ns].rearrange(
                        "p (s c r) -> p c s r", c=64, r=4)
                    dstc = ys_v[:, :, 16 * blk:16 * blk + 4 * ns].rearrange(
                        "p c (s r) -> p c s r", r=4)
                    if blk % 2 == 0:
                        nc.vector.tensor_copy(dstc, src)
                    else:
                        nc.scalar.copy(dstc, src)
                return ys

            def emit_s2_out(gb0, B, ys):
                if "s2" not in stages:
                    return
                ost = ostpool.tile([128, 32 * 260], F16, tag="ost")
                oc0 = 32 * gb0          # output col base for this group
                for q in range(32):
                    rhs_r = ys[:, B * q:B * q + B]
                    rhs_i = ys[:, B * (32 + q):B * (32 + q) + B]
                    ps2 = ps2pool.tile([128, 260], F32, tag="ps2")
                    nc.tensor.matmul(ps2[:, 0:B],
                                     t_gp[:, 128 * q:128 * q + 128],
                                     rhs_r, start=True, stop=False)
                    nc.tensor.matmul(ps2[:, 0:B],
                                     t_gq[:, 128 * q:128 * q + 128],
                                     rhs_i, start=False, stop=True)
                    d0 = ost[:, B * q:B * q + B]
                    if q % 2 == 0:
                        nc.vector.tensor_copy(d0, ps2[:, 0:B])
                    else:
                        nc.scalar.copy(d0, ps2[:, 0:B])
                    if "out" in stages and q in (7, 15, 23):
                        a = 8 * B * (q // 8)
                        nc.scalar.dma_start(
                            out.ap()[:, oc0 + a:oc0 + a + 8 * B],
                            ost[:, a:a + 8 * B])

                # bin 2048 (k1=0, k2=64)
                pse = psepool.tile([2, 260], F32, tag="pse")
                nc.tensor.matmul(pse[:, 0:B], t_e1[:], ys[:, 0:B],
                                 start=True, stop=False)
                nc.tensor.matmul(pse[:, 0:B], t_e2[:], ys[:, 32 * B:33 * B],
                                 start=False, stop=True)
                oste = ostpool.tile([2, 260], F16, tag="oste")
                nc.vector.tensor_copy(oste[:, 0:B], pse[:, 0:B])

                if "out" in stages:
                    nc.scalar.dma_start(
                        out.ap()[:, oc0 + 24 * B:oc0 + 32 * B],
                        ost[:, 24 * B:32 * B])
                    nc.scalar.dma_start(oute.ap()[:, gb0:gb0 + B],
                                        oste[:, 0:B])

            pending = None
            for gi, B in enumerate(GROUPS):
                ys = emit_load_s1(starts[gi], B)
                if pending is not None:
                    emit_s2_out(*pending)
                pending = (starts[gi], B, ys)
            emit_s2_out(*pending)

    nc.compile()
    return nc


def _prep_inputs(x, window):
    """Per-core stage-1 lhsT tensors: xfr[2, 128, 129*128] fp16 with
    partition p = 32j+8r+i holding frame-quarter j of frame 4*sg+r,
    cols = 128*sg + m, value = xp[1024*(b+j)+128i+m] * w[1024j+128i+m]."""
    pad = N_FFT // 2
    xp = np.pad(np.asarray(x), ((0, 0), (pad, pad)), mode="reflect")
    total = xp.shape[1]
    need = (NCORES - 1) * 512 * HOP + L
    xp_ext = np.zeros((2, max(total, need)), np.float32)
    xp_ext[:, :total] = xp
    w = np.asarray(window, np.float32)

    xfrs = []
    for i in range(NCORES):
        s0 = i * 512 * HOP
        seg = xp_ext[:, s0:s0 + L]
        xfr = np.empty((2, 128, NSG * 128), np.float16)
        for c in range(2):
            for j in range(4):
                Q = seg[c, 1024 * j:1024 * j + 1024 * NF].reshape(NF, 1024)
                Q = Q * w[1024 * j:1024 * (j + 1)][None, :]
                # [f, 1024] -> [sg, r, i, m] -> [r, i, sg, m]
                Q = Q.reshape(NSG, 4, 8, 128).transpose(1, 2, 0, 3)
                xfr[c, 32 * j:32 * j + 32] = \
                    Q.reshape(32, NSG * 128).astype(np.float16)
        xfrs.append(xfr)
    return xfrs


def kernel(x, window):
    import time
    t0 = time.time()
    x = np.asarray(x, np.float32)
    window = np.asarray(window, np.float32)
    if "nc" not in _cache:
        _cache["nc"] = _build()
    nc = _cache["nc"]
    print(f"[kernel] build done {time.time()-t0:.2f}s", flush=True)

    xfrs = _prep_inputs(x, window)
    R1D, R2D, Gp, Gq, E1, E2 = _host_constants()

    in_maps = []
    for i in range(NCORES):
        in_maps.append({"xfr": xfrs[i], "r1d": R1D, "r2d": R2D,
                        "gp": Gp, "gq": Gq, "e1": E1, "e2": E2})

    print(f"[kernel] inputs prepped {time.time()-t0:.2f}s", flush=True)
    res = bass_utils.run_bass_kernel_spmd(nc, in_maps,
                                          core_ids=list(range(NCORES)))
    print(f"[kernel] spmd done {time.time()-t0:.2f}s", flush=True)
    global LAST_EXEC_NS, LAST_RES
    LAST_RES = res
    if res.exec_time_ns is not None:
        LAST_EXEC_NS = res.exec_time_ns
        print(f"[kernel] exec_time_ns={res.exec_time_ns}", flush=True)
        if res.instructions_and_trace is not None:
            print(f"[kernel] trace={res.instructions_and_trace[1]}",
                  flush=True)

    out = np.zeros((2, NBINS, F_TOTAL), np.float32)
    for i in range(NCORES):
        o = res.results[i]["o"]            # [128, 32*NF] fp16
        oe = res.results[i]["oe"]          # [2, NF] fp16
        f0 = 512 * i
        nf = 513 if i == NCORES - 1 else 512
        full = np.empty((2, 2048, NF), np.float32)
        gb0 = 0
        for B in GROUPS:
            seg = o[:, 32 * gb0:32 * gb0 + 32 * B].astype(np.float32)
            # [128, 32*B] -> [c, p, q, b] -> [c, 32p+q, b]
            full[:, :, gb0:gb0 + B] = \
                seg.reshape(2, 64, 32, B).reshape(2, 2048, B)
            gb0 += B
        out[:, :2048, f0:f0 + nf] = full[:, :, :nf]
        out[:, 2048, f0:f0 + nf] = oe[:, :nf].astype(np.float32)
    return out
